# revision 1
# baseline (speedup 1.0000x reference)
"""Trainium2 Bass kernel for DCNv2 block (deformable conv + BN + exact GELU).

Problem: x[8,768,32,32] -> offset/mask 3x3 convs -> deformable 3x3 conv
(768->768, bilinear sampling, sigmoid-mask modulation) -> BatchNorm(batch
stats) -> exact GELU.

Strategy (8 NeuronCores, data-parallel over batch, BN stats all-reduced):
  per core (1 image):
    1. offset+mask conv as [j-on-partition, 27] PSUM-accumulated matmuls
       (stationary = shifted x windows) -> offT direct, no transposes
    2. bilinear corner indices + weights on DVE (clamping, border validity
       and the 2*sigmoid mask folded into 4 slot weights)
    3. v-GEMM per tap: v_k[j, co] = sum_c x[c, j] W[co, c, k] in bf16 with
       f32 PSUM; evac quantizes to int8 with one global adaptive scale
       (absmax pre-pass; the scale cancels exactly through batch-stat BN)
    4. int8 tables -> DRAM; SWDGE window-gathers (x-corner pairs) halve
       the gather traffic vs bf16; 4-corner combines run as acc-folded
       scalar_tensor_tensor chains accumulating y in f16 SBUF, split
       across Pool (3 j-groups) and ACT+DVE (5 j-groups)
    5. BN stats via ones-stationary matmuls -> [1, 1536], 8-core
       AllReduce, coefficients on one partition, broadcast, BN apply +
       exact GELU, out^T [1024,768] f32; host reassembles [8,768,32,32].
"""

import sys
import types

import numpy as np
import ml_dtypes

# Older axon client builds lack antenv.axon_hooks, which
# run_bass_kernel_spmd imports when tracing is requested via env. Stub it
# so the no-trace path always works standalone.
try:
    import antenv.axon_hooks  # noqa: F401
except ImportError:
    try:
        import antenv
        _stub = types.ModuleType("antenv.axon_hooks")
        _stub.get_axon_ntff_profile_hook = lambda: None
        sys.modules["antenv.axon_hooks"] = _stub
        antenv.axon_hooks = _stub
    except ImportError:
        pass

import bass_rust
import concourse.bass as bass
import concourse.mybir as mybir
import concourse.tile as tile
from concourse import bacc
from concourse.masks import make_identity
from concourse.bass_utils import run_bass_kernel_spmd

FP32 = mybir.dt.float32
BF16 = mybir.dt.bfloat16
F16 = mybir.dt.float16
I8 = mybir.dt.int8
I16 = mybir.dt.int16
AF = mybir.ActivationFunctionType
OP = mybir.AluOpType

B, C, H, W = 8, 768, 32, 32
CO, KS = 768, 3
K9 = KS * KS
HWN = H * W          # 1024
CT = C // 128        # 6 c-tiles
NG = HWN // 128      # 8 j-groups
MAX_OFF = float(min(H, W) // 4)  # 8.0

# per-(g,k) combine lane assignment (balances DVE/ACT/Pool load):
#  E : ACT scales A,C; DVE folds B,D + merges; Pool accumulates
#  A : ACT scales A,C; DVE folds B,D; Pool merges + accumulates
#  B : DVE-only acc-folded chain
# base per-g tap pattern, rotated by g to spread engine pressure in time
LANE_BASE = ("E", "A", "E", "B", "E", "A", "E", "B", "E")

# idx relayout staging geometry (as baseline)
NLIST = K9 * 2            # 18 gather index lists (tap x y-corner)
LIST_STRIDE = 8192        # int16 elems per list in DRAM (64 rows of 128)
RELAY_NIDX = 1152         # 18*64 rows covered by the relayout transpose-gather
IDXDRAM_LEN = 163840      # 1280 rows of 128, padded for relayout overreads

SCALE_MARGIN = 1.30       # int8 range headroom over the pre-pass absmax
DEBUG_DUMP = False


def build_nc(n_cores: int, phase: int = 9, no_coll: bool = False):
    nc = bacc.Bacc(None, target_bir_lowering=False, debug=False)

    x_in = nc.dram_tensor("x_bf", [CT, 128, HWN], BF16, kind="ExternalInput")
    wconv_in = nc.dram_tensor("wconv", [CT, 128, K9, 27], BF16, kind="ExternalInput")
    wproj_in = nc.dram_tensor("wproj", [CT, 128, K9, CO], BF16, kind="ExternalInput")
    bias27_in = nc.dram_tensor("bias27", [1, 27], FP32, kind="ExternalInput")
    gamma_in = nc.dram_tensor("gamma", [1, CO], FP32, kind="ExternalInput")
    beta_in = nc.dram_tensor("beta", [1, CO], FP32, kind="ExternalInput")
    out_t = nc.dram_tensor("out", [HWN, CO], FP32, kind="ExternalOutput")
    dbg = None
    if DEBUG_DUMP:
        dbg = {
            "d_offT": nc.dram_tensor("d_offT", [128, NG * 27], FP32,
                                     kind="ExternalOutput"),
            "d_scal": nc.dram_tensor("d_scal", [128, 1], FP32,
                                     kind="ExternalOutput"),
            "d_yacc": nc.dram_tensor("d_yacc", [128, NG * CO], FP32,
                                     kind="ExternalOutput"),
            "d_stats": nc.dram_tensor("d_stats", [1, 2 * CO], FP32,
                                      kind="ExternalOutput"),
            "d_vq0": nc.dram_tensor("d_vq0", [HWN, CO], FP32,
                                    kind="ExternalOutput"),
        }

    from contextlib import ExitStack
    with tile.TileContext(nc) as tc:
        with ExitStack() as ctx:
            _emit(ctx, tc, n_cores, x_in, wconv_in, wproj_in, bias27_in,
                  gamma_in, beta_in, out_t, phase, no_coll, dbg)
    nc.compile()
    return nc


def _emit(ctx, tc, n_cores, x_in, wconv_in, wproj_in, bias27_in,
          gamma_in, beta_in, out_t, phase=9, no_coll=False, dbg=None):
    nc = tc.nc

    cpool = ctx.enter_context(tc.tile_pool(name="consts", bufs=1))
    wpool = ctx.enter_context(tc.tile_pool(name="weights", bufs=1))
    kpool = ctx.enter_context(tc.tile_pool(name="wk", bufs=2))
    spool = ctx.enter_context(tc.tile_pool(name="scratch", bufs=1))
    gpool = ctx.enter_context(tc.tile_pool(name="gather", bufs=2))
    tpool = ctx.enter_context(tc.tile_pool(name="tcomb", bufs=4))
    ypool = ctx.enter_context(tc.tile_pool(name="ybuf", bufs=1))
    opool = ctx.enter_context(tc.tile_pool(name="outb", bufs=2))
    vpool = ctx.enter_context(tc.tile_pool(name="vq", bufs=2))
    dpool = ctx.enter_context(tc.tile_pool(name="dram", bufs=1, space="DRAM"))
    from contextlib import ExitStack as _ES
    early_ps = _ES()
    pp_small = early_ps.enter_context(tc.tile_pool(name="ps_small", bufs=2,
                                                   space="PSUM"))

    # ---------------- constants ----------------
    ident_f = cpool.tile([128, 128], FP32, tag="identf", name="identf")
    make_identity(nc, ident_f[:, :])
    ones_b = cpool.tile([128, 1], BF16, tag="onesb", name="onesb")
    nc.vector.memset(ones_b[:, :], 1.0)
    ones_h = cpool.tile([128, 1], F16, tag="onesh", name="onesh")
    nc.vector.memset(ones_h[:, :], 1.0)

    pf_i = cpool.tile([128, 1], mybir.dt.int32, tag="pfi", name="pfi")
    nc.gpsimd.iota(pf_i[:, :], pattern=[[0, 1]], base=0, channel_multiplier=1)
    pf = cpool.tile([128, 1], FP32, tag="pf", name="pf")
    nc.vector.tensor_copy(pf[:, :], pf_i[:, :])
    # hdiv = p // 32 (exact, via 3 compares); pm32 = p % 32
    hdiv = cpool.tile([128, 1], FP32, tag="hdiv", name="hdiv")
    tmp1 = cpool.tile([128, 1], FP32, tag="tmp1", name="tmp1")
    nc.vector.tensor_scalar(hdiv[:, :], pf[:, :], 32.0, None, OP.is_ge)
    nc.vector.tensor_scalar(tmp1[:, :], pf[:, :], 64.0, None, OP.is_ge)
    nc.vector.tensor_tensor(hdiv[:, :], hdiv[:, :], tmp1[:, :], OP.add)
    nc.vector.tensor_scalar(tmp1[:, :], pf[:, :], 96.0, None, OP.is_ge)
    nc.vector.tensor_tensor(hdiv[:, :], hdiv[:, :], tmp1[:, :], OP.add)
    pm32 = cpool.tile([128, 1], FP32, tag="pm32", name="pm32")
    nc.vector.scalar_tensor_tensor(pm32[:, :], hdiv[:, :], -32.0, pf[:, :],
                                   OP.mult, OP.add)

    kyrow_i = cpool.tile([128, K9], mybir.dt.int32, tag="kyrowi", name="kyrowi")
    nc.gpsimd.iota(kyrow_i[:, :].rearrange("p (a b) -> p a b", a=3),
                   pattern=[[1, 3], [0, 3]], base=0, channel_multiplier=0)
    kyrow = cpool.tile([128, K9], FP32, tag="kyrow", name="kyrow")
    nc.vector.tensor_copy(kyrow[:, :], kyrow_i[:, :])
    kxrow_i = cpool.tile([128, K9], mybir.dt.int32, tag="kxrowi", name="kxrowi")
    nc.gpsimd.iota(kxrow_i[:, :].rearrange("p (a b) -> p a b", a=3),
                   pattern=[[0, 3], [1, 3]], base=0, channel_multiplier=0)
    kxrow = cpool.tile([128, K9], FP32, tag="kxrow", name="kxrow")
    nc.vector.tensor_copy(kxrow[:, :], kxrow_i[:, :])

    # basex[p,k] = (p%32) + kx - 1   (same for every j-group)
    basex = cpool.tile([128, K9], FP32, tag="basex", name="basex")
    nc.vector.tensor_scalar(basex[:, :], kxrow[:, :], pm32[:, :], -1.0,
                            OP.add, OP.add)
    # basey[p,g,k] = (p//32) + 4g + ky - 1
    basey = cpool.tile([128, NG, K9], FP32, tag="basey", name="basey")
    for g in range(NG):
        nc.vector.tensor_scalar(basey[:, g, :], kyrow[:, :], hdiv[:, :],
                                float(4 * g - 1), OP.add, OP.add)

    # relayout-gather identity index list: value 16*s + (p % 16)
    pm16 = cpool.tile([128, 1], FP32, tag="pm16", name="pm16")
    nc.vector.tensor_scalar(pm16[:, :], pm32[:, :], 16.0, None, OP.is_ge)
    nc.vector.scalar_tensor_tensor(pm16[:, :], pm16[:, :], -16.0, pm32[:, :],
                                   OP.mult, OP.add)
    relay_i32 = cpool.tile([128, RELAY_NIDX // 16], mybir.dt.int32,
                           tag="relayi32", name="relayi32")
    nc.gpsimd.iota(relay_i32[:, :], pattern=[[16, RELAY_NIDX // 16]], base=0,
                   channel_multiplier=0)
    relay_f = cpool.tile([128, RELAY_NIDX // 16], FP32, tag="relayf", name="relayf")
    nc.vector.tensor_copy(relay_f[:, :], relay_i32[:, :])
    nc.vector.tensor_scalar(relay_f[:, :], relay_f[:, :], pm16[:, :], None, OP.add)
    relay_idx = cpool.tile([128, RELAY_NIDX // 16], I16, tag="relayidx", name="relayidx")
    nc.vector.tensor_copy(relay_idx[:, :], relay_f[:, :])

    # ---------------- load inputs ----------------
    x_sb = wpool.tile([128, CT, HWN], BF16, tag="xsb", name="xsb")
    nc.sync.dma_start(
        x_sb[:, :, :],
        x_in[:, :, :].rearrange("c p n -> p c n"))
    wconv_sb = wpool.tile([128, CT, K9, 27], BF16, tag="wconvsb", name="wconvsb")
    nc.sync.dma_start(
        wconv_sb[:, :, :, :],
        wconv_in[:, :, :, :].rearrange("c p k m -> p c k m"))
    bias27_sb = wpool.tile([1, 27], FP32, tag="bias27", name="bias27")
    nc.sync.dma_start(bias27_sb[:, :], bias27_in[:, :])
    gamma_sb = wpool.tile([1, CO], FP32, tag="gammasb", name="gammasb")
    nc.sync.dma_start(gamma_sb[:, :], gamma_in[:, :])
    beta_sb = wpool.tile([1, CO], FP32, tag="betasb", name="betasb")
    nc.sync.dma_start(beta_sb[:, :], beta_in[:, :])

    # conv bias as [27, 1] column (per-partition bias for the conv evac)
    bias27_col = wpool.tile([27, 1], FP32, tag="bias27col", name="bias27col")
    nc.sync.dma_start(bias27_col[:, :], bias27_in[:, :].rearrange("q k -> k q"))

    # ---------------- zero-padded x (34x34) for conv windows ----------------
    from contextlib import ExitStack
    early = ExitStack()
    xpool = early.enter_context(tc.tile_pool(name="xpadp", bufs=1))
    xpad = xpool.tile([128, CT, 34 * 34], BF16, tag="xpad", name="xpad")
    nc.vector.memset(xpad[:, :, :], 0.0)
    for ct in range(CT):
        nc.vector.tensor_copy(
            xpad[:, ct, :].rearrange("p (h w) -> p h w", h=34)[:, 1:33, 1:33],
            x_sb[:, ct, :].rearrange("p (h w) -> p h w", h=H))
    xp_im = [xpad[:, ct, :].rearrange("p (h w) -> p h w", h=34) for ct in range(CT)]
    shifts = [(dy, dx) for dy in (-1, 0, 1) for dx in (-1, 0, 1)]

    # ---------------- offset/mask conv (27 out-ch), baseline orientation ----
    offs_nat = spool.tile([27, HWN], FP32, tag="offsnat", name="offsnat")
    for half in range(2):
        conv_ps = pp_small.tile([27, 512], FP32, tag="convps", name="convps")
        first = True
        for dy, dx in shifts:
            s = (dy + 1) * 3 + (dx + 1)
            for ct in range(CT):
                nc.tensor.matmul(
                    conv_ps[:, :],
                    wconv_sb[:, ct, s, :],
                    xp_im[ct][:, 1 + dy + 16 * half:1 + dy + 16 * half + 16,
                              1 + dx:1 + dx + 32],
                    start=first, stop=(dy, dx) == shifts[-1] and ct == CT - 1)
                first = False
        nc.scalar.activation(
            offs_nat[:, 512 * half:512 * (half + 1)],
            conv_ps[:, :],
            AF.Identity, bias=bias27_col[:, :])
    early.close()  # free xpad

    # transpose to j-on-partition: offT [128, g, 27]
    offT = spool.tile([128, NG, 27], FP32, tag="offT", name="offT")
    for g in range(NG):
        tp27 = pp_small.tile([128, 32], FP32, tag="tp27", name="tp27")
        nc.tensor.transpose(tp27[:, 0:27], offs_nat[:, g * 128:(g + 1) * 128],
                            ident_f[:27, :27])
        nc.scalar.activation(offT[:, g, :], tp27[:, 0:27], AF.Copy)

    if dbg is not None:
        nc.sync.dma_start(dbg["d_offT"][:, :],
                          offT[:, :, :].rearrange("p g k -> p (g k)"))

    # ---------------- bilinear indices + weights ----------------
    def f3(tag):
        return spool.tile([128, NG, K9], FP32, tag=tag, name=tag)

    offy = offT[:, :, 0:18:2]
    offx = offT[:, :, 1:18:2]
    py = f3("py")
    px = f3("px")
    nc.vector.tensor_scalar(py[:, :, :], offy, -MAX_OFF, MAX_OFF, OP.max, OP.min)
    nc.vector.tensor_tensor(py[:, :, :], py[:, :, :], basey[:, :, :], OP.add)
    nc.vector.tensor_scalar(px[:, :, :], offx, -MAX_OFF, MAX_OFF, OP.max, OP.min)
    for g in range(NG):
        nc.vector.tensor_tensor(px[:, g, :], px[:, g, :], basex[:, :], OP.add)

    # robust floor via int cast (works for trunc or round-to-nearest)
    def fl(src, tag):
        t = f3(tag + "_t")
        nc.vector.tensor_scalar(t[:, :, :], src[:, :, :], 16.0, None, OP.add)
        ti = spool.tile([128, NG, K9], I16, tag=tag + "_i", name=tag + "_i")
        nc.vector.tensor_copy(ti[:, :, :], t[:, :, :])
        cf = f3(tag + "_cf")
        nc.vector.tensor_copy(cf[:, :, :], ti[:, :, :])
        over = f3(tag + "_ov")
        nc.vector.tensor_tensor(over[:, :, :], cf[:, :, :], t[:, :, :], OP.is_gt)
        nc.vector.tensor_tensor(cf[:, :, :], cf[:, :, :], over[:, :, :], OP.subtract)
        y0 = f3(tag + "_y0")
        nc.vector.tensor_scalar(y0[:, :, :], cf[:, :, :], 16.0, None, OP.subtract)
        fr = f3(tag + "_fr")
        nc.vector.tensor_tensor(fr[:, :, :], src[:, :, :], y0[:, :, :], OP.subtract)
        return y0, fr

    y0, fy = fl(py, "fy")
    x0, fx = fl(px, "fx")

    y0c = f3("y0c")
    nc.vector.tensor_scalar(y0c[:, :, :], y0[:, :, :], 0.0, 31.0, OP.max, OP.min)
    y1 = f3("y1")
    nc.vector.tensor_scalar(y1[:, :, :], y0[:, :, :], 1.0, None, OP.add)
    y1c = f3("y1c")
    nc.vector.tensor_scalar(y1c[:, :, :], y1[:, :, :], 0.0, 31.0, OP.max, OP.min)
    x0c = f3("x0c")
    nc.vector.tensor_scalar(x0c[:, :, :], x0[:, :, :], 0.0, 30.0, OP.max, OP.min)

    vy0 = f3("vy0")
    nc.vector.tensor_tensor(vy0[:, :, :], y0c[:, :, :], y0[:, :, :], OP.is_equal)
    vy1 = f3("vy1")
    nc.vector.tensor_tensor(vy1[:, :, :], y1c[:, :, :], y1[:, :, :], OP.is_equal)

    # x validity of corners A (x0) and B (x0+1)
    vxA = f3("vxA")
    t2 = f3("t2")
    nc.vector.tensor_scalar(vxA[:, :, :], x0[:, :, :], 0.0, None, OP.is_ge)
    nc.vector.tensor_scalar(t2[:, :, :], x0[:, :, :], 31.0, None, OP.is_le)
    nc.vector.tensor_tensor(vxA[:, :, :], vxA[:, :, :], t2[:, :, :], OP.mult)
    vxB = f3("vxB")
    nc.vector.tensor_scalar(vxB[:, :, :], x0[:, :, :], -1.0, None, OP.is_ge)
    nc.vector.tensor_scalar(t2[:, :, :], x0[:, :, :], 30.0, None, OP.is_le)
    nc.vector.tensor_tensor(vxB[:, :, :], vxB[:, :, :], t2[:, :, :], OP.mult)

    # slot coverage: slot0 = x0c, slot1 = x0c+1
    eqA = f3("eqA")
    nc.vector.tensor_tensor(eqA[:, :, :], x0c[:, :, :], x0[:, :, :], OP.is_equal)
    e0B = f3("e0B")
    nc.vector.tensor_scalar(t2[:, :, :], x0c[:, :, :], 1.0, None, OP.subtract)
    nc.vector.tensor_tensor(e0B[:, :, :], t2[:, :, :], x0[:, :, :], OP.is_equal)
    e1A = f3("e1A")
    nc.vector.tensor_scalar(t2[:, :, :], x0[:, :, :], 1.0, None, OP.subtract)
    nc.vector.tensor_tensor(e1A[:, :, :], x0c[:, :, :], t2[:, :, :], OP.is_equal)

    # a = (1-fx)*vxA ; b = fx*vxB
    wa = f3("wa")
    nc.vector.tensor_scalar(wa[:, :, :], fx[:, :, :], 1.0, -1.0, OP.subtract, OP.mult)
    nc.vector.tensor_tensor(wa[:, :, :], wa[:, :, :], vxA[:, :, :], OP.mult)
    wb = f3("wb")
    nc.vector.tensor_tensor(wb[:, :, :], fx[:, :, :], vxB[:, :, :], OP.mult)
    ws0 = f3("ws0")
    nc.vector.tensor_tensor(ws0[:, :, :], wa[:, :, :], eqA[:, :, :], OP.mult)
    nc.vector.tensor_tensor(t2[:, :, :], wb[:, :, :], e0B[:, :, :], OP.mult)
    nc.vector.tensor_tensor(ws0[:, :, :], ws0[:, :, :], t2[:, :, :], OP.add)
    ws1 = f3("ws1")
    nc.vector.tensor_tensor(ws1[:, :, :], wa[:, :, :], e1A[:, :, :], OP.mult)
    nc.vector.tensor_tensor(t2[:, :, :], wb[:, :, :], eqA[:, :, :], OP.mult)
    nc.vector.tensor_tensor(ws1[:, :, :], ws1[:, :, :], t2[:, :, :], OP.add)

    # y weights with 2*sigmoid(mask) folded in
    sig = f3("sig")
    nc.scalar.activation(sig[:, :, :], offT[:, :, 18:27], AF.Sigmoid)
    wy0 = f3("wy0")
    nc.vector.tensor_scalar(wy0[:, :, :], fy[:, :, :], 1.0, -2.0, OP.subtract, OP.mult)
    nc.vector.tensor_tensor(wy0[:, :, :], wy0[:, :, :], sig[:, :, :], OP.mult)
    nc.vector.tensor_tensor(wy0[:, :, :], wy0[:, :, :], vy0[:, :, :], OP.mult)
    wy1 = f3("wy1")
    nc.vector.tensor_scalar(wy1[:, :, :], fy[:, :, :], 2.0, None, OP.mult)
    nc.vector.tensor_tensor(wy1[:, :, :], wy1[:, :, :], sig[:, :, :], OP.mult)
    nc.vector.tensor_tensor(wy1[:, :, :], wy1[:, :, :], vy1[:, :, :], OP.mult)

    w00 = f3("w00")
    w01 = f3("w01")
    w10 = f3("w10")
    w11 = f3("w11")
    nc.vector.tensor_tensor(w00[:, :, :], wy0[:, :, :], ws0[:, :, :], OP.mult)
    nc.vector.tensor_tensor(w01[:, :, :], wy0[:, :, :], ws1[:, :, :], OP.mult)
    nc.vector.tensor_tensor(w10[:, :, :], wy1[:, :, :], ws0[:, :, :], OP.mult)
    nc.vector.tensor_tensor(w11[:, :, :], wy1[:, :, :], ws1[:, :, :], OP.mult)

    # flat indices (rows of the v tables), int16, staged [128, g, list]
    idlo = f3("idlo")
    nc.vector.scalar_tensor_tensor(idlo[:, :, :], y0c[:, :, :], 32.0,
                                   x0c[:, :, :], OP.mult, OP.add)
    idhi = f3("idhi")
    nc.vector.scalar_tensor_tensor(idhi[:, :, :], y1c[:, :, :], 32.0,
                                   x0c[:, :, :], OP.mult, OP.add)
    idfl = spool.tile([128, NG, NLIST], FP32, tag="idfl", name="idfl")
    nc.vector.tensor_copy(idfl[:, :, 0:NLIST:2], idlo[:, :, :])
    nc.vector.tensor_copy(idfl[:, :, 1:NLIST:2], idhi[:, :, :])
    # transpose to list-on-partition: S[l, 128g + p] = idx(p, g, l)
    s_f = spool.tile([NLIST, HWN], FP32, tag="sfidx", name="sfidx")
    for g in range(NG):
        tpx2 = pp_small.tile([NLIST, 128], FP32, tag="tpx2", name="tpx2")
        nc.tensor.transpose(tpx2[:, :], idfl[:, g, :], ident_f[:, :])
        nc.scalar.activation(s_f[:, g * 128:(g + 1) * 128], tpx2[:, :], AF.Copy)
    s_i = spool.tile([NLIST, HWN], I16, tag="siidx", name="siidx")
    nc.vector.tensor_copy(s_i[:, :], s_f[:, :])
    early_ps.close()  # free conv/transpose PSUM banks

    if phase <= 1:
        return
    # ---------------- idx relayout through DRAM ----------------
    idxdram = dpool.tile([IDXDRAM_LEN], I16, tag="idxdram", name="idxdram")
    zt = spool.tile([128, IDXDRAM_LEN // 128], I16, tag="zeros", name="zeros")
    nc.vector.memset(zt[:, :], 0)
    nc.gpsimd.dma_start(idxdram[:IDXDRAM_LEN].rearrange("(a b) -> a b", a=128),
                        zt[:, :])
    # scatter stage -> flat j-order per list: dram[(l*64+s)*128 + 16*rep + r]
    _idxd = idxdram[:]
    for rep in range(8):
        dst = bass.AP(_idxd.tensor, 16 * rep,
                      [[LIST_STRIDE, NLIST], [128, 64], [1, 16]])
        nc.gpsimd.dma_start(dst, s_i[:, :])
    idxall = spool.tile([128, RELAY_NIDX], I16, tag="idxall", name="idxall")
    relay_src = idxdram[:].bitcast(BF16)
    relay_src = bass.AP(relay_src.tensor, relay_src.offset,
                        [[128, IDXDRAM_LEN // 128], [1, 128]])
    nc.gpsimd.dma_gather(
        out_ap=idxall[:, :].bitcast(BF16).rearrange("p (a b) -> p a b", a=1),
        in_ap=relay_src,
        idxs_ap=relay_idx[:, :],
        num_idxs=RELAY_NIDX,
        num_idxs_reg=RELAY_NIDX,
        elem_size=128,
        transpose=True,
        single_packet=False,
    )

    if phase <= 2:
        return

    # ---------------- int8 scale pre-pass (tap 0, jc 0..1) ----------------
    pp_gemm = ctx.enter_context(tc.tile_pool(name="ps_gemm", bufs=2,
                                             space="PSUM"))
    wk0 = kpool.tile([128, CT, CO], BF16, tag="wk", name="wk0")
    nc.sync.dma_start(wk0[:, :, :],
                      wproj_in[:, :, 0, :].rearrange("c p n -> p c n"))
    pmax = spool.tile([128, 2], FP32, tag="pmax", name="pmax")
    for jc in range(2):
        ps = pp_gemm.tile([128, CO], FP32, tag="gps", name="ppre")
        for ct in range(CT):
            nc.tensor.matmul(ps[:, 0:512], x_sb[:, ct, jc * 128:(jc + 1) * 128],
                             wk0[:, ct, 0:512], start=ct == 0, stop=ct == CT - 1)
        for ct in range(CT):
            nc.tensor.matmul(ps[:, 512:768], x_sb[:, ct, jc * 128:(jc + 1) * 128],
                             wk0[:, ct, 512:768], start=ct == 0, stop=ct == CT - 1)
        nc.vector.reduce_max(pmax[:, jc:jc + 1], ps[:, :],
                             axis=mybir.AxisListType.X,
                             apply_absolute_value=True)
    gmax = spool.tile([128, 1], FP32, tag="gmax", name="gmax")
    nc.vector.reduce_max(gmax[:, :], pmax[:, :], axis=mybir.AxisListType.X,
                         apply_absolute_value=True)
    gmax_all = spool.tile([128, 1], FP32, tag="gmaxall", name="gmaxall")
    nc.gpsimd.partition_all_reduce(gmax_all[:, :], gmax[:, :], channels=128,
                                   reduce_op=bass_rust.ReduceOp.max)
    # scal = 126.5 / (margin * gmax)
    scal = spool.tile([128, 1], FP32, tag="scal", name="scal")
    nc.vector.tensor_scalar(scal[:, :], gmax_all[:, :], SCALE_MARGIN / 126.5,
                            None, OP.mult)
    nc.vector.reciprocal(scal[:, :], scal[:, :])
    if dbg is not None:
        nc.sync.dma_start(dbg["d_scal"][:, :], scal[:, :])

    if phase <= 3:
        return

    # ---------------- v-GEMM + int8 tables + fused sampling ----------------
    vtabs = [dpool.tile([HWN, CO], I8, tag=f"vtab{k}", name=f"vtab{k}")
             for k in range(K9)]
    y_acc = ypool.tile([128, NG, CO], F16, tag="yacc", name="yacc")

    def sample_tap(k):
        """Window gathers + 4-corner combines for tap k (all 8 groups)."""
        gts = []
        for yc in range(2):
            gt = gpool.tile([128, NG, 2 * CO], I8, tag=f"gt{yc}", name=f"gt{yc}",
                            bufs=2)
            lcol = (2 * k + yc) * 64
            vsrc = vtabs[k][:, :]
            vsrc = bass.AP(vsrc.tensor, vsrc.offset,
                           [[CO, HWN - 1], [1, 2 * CO]])
            nc.gpsimd.dma_gather(
                out_ap=gt[:, :, :],
                in_ap=vsrc,
                idxs_ap=idxall[:, lcol:lcol + 64],
                num_idxs=HWN,
                num_idxs_reg=HWN,
                elem_size=2 * CO,
                elem_step=CO,
                single_packet=False,
            )
            gts.append(gt)
        gt0, gt1 = gts
        for g in range(NG):
            A = gt0[:, g, 0:CO]
            Bc = gt0[:, g, CO:2 * CO]
            Cc = gt1[:, g, 0:CO]
            Dc = gt1[:, g, CO:2 * CO]
            wa = w00[:, g, k:k + 1]
            wb = w01[:, g, k:k + 1]
            wc = w10[:, g, k:k + 1]
            wd = w11[:, g, k:k + 1]
            acc = y_acc[:, g, :]
            lane = LANE_BASE[(k + g) % len(LANE_BASE)]
            t1 = tpool.tile([128, CO], F16, tag="tc1", name="tc1", bufs=4)
            t2 = tpool.tile([128, CO], F16, tag="tc2", name="tc2", bufs=4)
            if lane == "B":
                # DVE-only acc-folded chain
                if k == 0:
                    nc.vector.tensor_scalar(t1[:, :], A, wa, None, OP.mult)
                else:
                    nc.vector.scalar_tensor_tensor(t1[:, :], A, wa, acc,
                                                   OP.mult, OP.add)
                nc.vector.scalar_tensor_tensor(t1[:, :], Bc, wb, t1[:, :],
                                               OP.mult, OP.add)
                nc.vector.scalar_tensor_tensor(t1[:, :], Cc, wc, t1[:, :],
                                               OP.mult, OP.add)
                nc.vector.scalar_tensor_tensor(acc, Dc, wd, t1[:, :],
                                               OP.mult, OP.add)
            else:
                nc.scalar.activation(t1[:, :], A, AF.Copy, scale=wa)
                nc.scalar.activation(t2[:, :], Cc, AF.Copy, scale=wc)
                nc.vector.scalar_tensor_tensor(t1[:, :], Bc, wb, t1[:, :],
                                               OP.mult, OP.add)
                nc.vector.scalar_tensor_tensor(t2[:, :], Dc, wd, t2[:, :],
                                               OP.mult, OP.add)
                if lane == "E":
                    nc.vector.tensor_tensor(t1[:, :], t1[:, :], t2[:, :], OP.add)
                    if k == 0:
                        nc.gpsimd.tensor_copy(acc, t1[:, :])
                    else:
                        nc.gpsimd.tensor_tensor(acc, acc, t1[:, :], OP.add)
                else:  # A
                    nc.gpsimd.tensor_tensor(t1[:, :], t1[:, :], t2[:, :], OP.add)
                    if k == 0:
                        nc.gpsimd.tensor_copy(acc, t1[:, :])
                    else:
                        nc.gpsimd.tensor_tensor(acc, acc, t1[:, :], OP.add)

    for k in range(K9):
        if k == 0:
            wk = wk0
        else:
            wk = kpool.tile([128, CT, CO], BF16, tag="wk", name=f"wk{k}")
            nc.sync.dma_start(wk[:, :, :],
                              wproj_in[:, :, k, :].rearrange("c p n -> p c n"))
        vstage = vpool.tile([128, NG, CO], I8, tag="vstage", name="vstage",
                            bufs=2)
        for jc in range(NG):
            ps = pp_gemm.tile([128, CO], FP32, tag="gps", name="gps")
            for ct in range(CT):
                nc.tensor.matmul(
                    ps[:, 0:512],
                    x_sb[:, ct, jc * 128:(jc + 1) * 128],
                    wk[:, ct, 0:512],
                    start=ct == 0, stop=ct == CT - 1)
            for ct in range(CT):
                nc.tensor.matmul(
                    ps[:, 512:768],
                    x_sb[:, ct, jc * 128:(jc + 1) * 128],
                    wk[:, ct, 512:768],
                    start=ct == 0, stop=ct == CT - 1)
            # quantized evac: int8 = round(v * scal)
            nc.scalar.activation(vstage[:, jc, :], ps[:, :], AF.Copy,
                                 scale=scal[:, :])
        vdst = vtabs[k][:, :]
        vdst = bass.AP(vdst.tensor, vdst.offset,
                       [[CO, 128], [128 * CO, NG], [1, CO]])
        nc.sync.dma_start(vdst, vstage[:, :, :])
        if dbg is not None and k == 0:
            for jc in range(NG):
                dt_ = opool.tile([128, CO], FP32, tag="og", name="dvq")
                nc.vector.tensor_copy(dt_[:, :], vstage[:, jc, :])
                nc.sync.dma_start(dbg["d_vq0"][jc * 128:(jc + 1) * 128, :],
                                  dt_[:, :])
        sample_tap(k)

    if phase <= 4:
        return

    # ---------------- BN stats: [1, 768] sums via ones-stationary ----------
    pp_stats = ctx.enter_context(tc.tile_pool(name="ps_stats", bufs=1,
                                              space="PSUM"))
    stats_y = [pp_stats.tile([1, 512], FP32, tag="sty0", name="sty0"),
               pp_stats.tile([1, 256], FP32, tag="sty1", name="sty1")]
    stats_q = [pp_stats.tile([1, 512], FP32, tag="stq0", name="stq0"),
               pp_stats.tile([1, 256], FP32, tag="stq1", name="stq1")]
    for g in range(NG):
        ysq = tpool.tile([128, CO], BF16, tag="ysq", name="ysq", bufs=2)
        nc.scalar.activation(ysq[:, :], y_acc[:, g, :], AF.Square)
        nc.tensor.matmul(stats_y[0][:, :], ones_h[:, :], y_acc[:, g, 0:512],
                         start=g == 0, stop=g == NG - 1)
        nc.tensor.matmul(stats_y[1][:, :], ones_h[:, :], y_acc[:, g, 512:768],
                         start=g == 0, stop=g == NG - 1)
        nc.tensor.matmul(stats_q[0][:, :], ones_b[:, :], ysq[:, 0:512],
                         start=g == 0, stop=g == NG - 1)
        nc.tensor.matmul(stats_q[1][:, :], ones_b[:, :], ysq[:, 512:768],
                         start=g == 0, stop=g == NG - 1)
    # convert this core's sums to TRUE units before the all-reduce
    # (y_acc = scal * y_true and scal differs per core): sums /= s, sq /= s^2
    sinv = spool.tile([1, 1], FP32, tag="sinv", name="sinv")
    nc.vector.reciprocal(sinv[:, :], scal[0:1, :])
    sinv2 = spool.tile([1, 1], FP32, tag="sinv2", name="sinv2")
    nc.vector.tensor_tensor(sinv2[:, :], sinv[:, :], sinv[:, :], OP.mult)
    stats_sb = spool.tile([1, 2 * CO], FP32, tag="statssb", name="statssb")
    nc.vector.tensor_scalar(stats_sb[:, 0:512], stats_y[0][:, :],
                            sinv[:, :], None, OP.mult)
    nc.vector.tensor_scalar(stats_sb[:, 512:768], stats_y[1][:, :],
                            sinv[:, :], None, OP.mult)
    nc.vector.tensor_scalar(stats_sb[:, 768:1280], stats_q[0][:, :],
                            sinv2[:, :], None, OP.mult)
    nc.vector.tensor_scalar(stats_sb[:, 1280:1536], stats_q[1][:, :],
                            sinv2[:, :], None, OP.mult)

    if dbg is not None:
        for g in range(NG):
            dty = opool.tile([128, CO], FP32, tag="og", name="dty")
            nc.vector.tensor_copy(dty[:, :], y_acc[:, g, :])
            nc.sync.dma_start(dbg["d_yacc"][:, g * CO:(g + 1) * CO], dty[:, :])
        nc.sync.dma_start(dbg["d_stats"][:, :], stats_sb[:, :])

    # ---------------- BN stats all-reduce + coefficients ----------------
    cc_in = dpool.tile([1, 2 * CO], FP32, tag="ccin", name="ccin")
    cc_out = dpool.tile([1, 2 * CO], FP32, tag="ccout", name="ccout")
    nc.gpsimd.dma_start(cc_in[:, :], stats_sb[:, :])
    if no_coll:
        nc.gpsimd.dma_start(cc_out[:, :], cc_in[:, :])
    else:
        nc.gpsimd.collective_compute(
            "AllReduce", OP.add,
            replica_groups=[list(range(n_cores))],
            ins=[cc_in[:, :].opt()],
            outs=[cc_out[:, :].opt()],
        )
    nc.gpsimd.dma_start(stats_sb[:, :], cc_out[:, :])

    n_inv = 1.0 / float(n_cores * HWN)
    mean = spool.tile([1, CO], FP32, tag="mean", name="mean")
    nc.vector.tensor_scalar(mean[:, :], stats_sb[:, 0:CO], n_inv, None, OP.mult)
    var = spool.tile([1, CO], FP32, tag="var", name="var")
    nc.vector.tensor_tensor(var[:, :], mean[:, :], mean[:, :], OP.mult)
    nc.vector.scalar_tensor_tensor(var[:, :], stats_sb[:, CO:2 * CO], n_inv,
                                   var[:, :], OP.mult, OP.subtract)
    nc.vector.tensor_scalar(var[:, :], var[:, :], 1e-5, None, OP.add)
    sc_row = spool.tile([1, CO], FP32, tag="scrow", name="scrow")
    nc.vector.reciprocal(sc_row[:, :], var[:, :])
    nc.scalar.sqrt(sc_row[:, :], sc_row[:, :])
    nc.vector.tensor_tensor(sc_row[:, :], sc_row[:, :], gamma_sb[:, :], OP.mult)
    sh_row = spool.tile([1, CO], FP32, tag="shrow", name="shrow")
    nc.vector.tensor_tensor(sh_row[:, :], mean[:, :], sc_row[:, :], OP.mult)
    nc.vector.tensor_tensor(sh_row[:, :], beta_sb[:, :], sh_row[:, :], OP.subtract)
    # apply reads y_acc (still in scaled units): fold 1/s into the scale coef
    nc.vector.tensor_scalar(sc_row[:, :], sc_row[:, :], sinv[:, :], None,
                            OP.mult)

    # broadcast [1, 1536] f16 coefficients to all partitions
    scsh_row = spool.tile([1, 2 * CO], F16, tag="scshrow", name="scshrow")
    nc.vector.tensor_copy(scsh_row[:, 0:CO], sc_row[:, :])
    nc.vector.tensor_copy(scsh_row[:, CO:2 * CO], sh_row[:, :])
    scsh = spool.tile([128, 2 * CO], F16, tag="scsh", name="scsh")
    nc.gpsimd.partition_broadcast(scsh[:, :], scsh_row[:1, :])
    sc_b = scsh[:, 0:CO]
    sh_b = scsh[:, CO:2 * CO]

    # ---------------- BN apply + GELU + out ----------------
    for g in range(NG):
        yb = tpool.tile([128, CO], F16, tag="yapply", name="yapply")
        if g % 2 == 0:
            nc.gpsimd.tensor_tensor(yb[:, :], y_acc[:, g, :], sc_b[:, :], OP.mult)
            nc.gpsimd.tensor_tensor(yb[:, :], yb[:, :], sh_b[:, :], OP.add)
        else:
            nc.vector.tensor_tensor(yb[:, :], y_acc[:, g, :], sc_b[:, :], OP.mult)
            nc.vector.tensor_tensor(yb[:, :], yb[:, :], sh_b[:, :], OP.add)
        og = opool.tile([128, CO], FP32, tag="og", name="og")
        nc.scalar.activation(og[:, :], yb[:, :], AF.Gelu)
        nc.sync.dma_start(out_t[g * 128:(g + 1) * 128, :], og[:, :])


_NC_CACHE = {}


def _get_nc(n_cores):
    if n_cores not in _NC_CACHE:
        _NC_CACHE[n_cores] = build_nc(n_cores)
    return _NC_CACHE[n_cores]


def prep_inputs(x, proj_w, proj_b, off_w, off_b, mask_w, mask_b, gamma, beta):
    """Build the per-core in_maps (host-side layout prep only)."""
    bf = ml_dtypes.bfloat16
    wconv = np.concatenate([np.asarray(off_w), np.asarray(mask_w)], axis=0)
    wconv = wconv.reshape(27, CT, 128, K9).transpose(1, 2, 3, 0).astype(bf)
    wproj = np.asarray(proj_w).reshape(CO, CT, 128, K9).transpose(1, 2, 3, 0).astype(bf)
    bias27 = np.concatenate([np.asarray(off_b), np.asarray(mask_b)]).reshape(1, 27)
    bias27 = np.ascontiguousarray(bias27, dtype=np.float32)
    ga = np.ascontiguousarray(np.asarray(gamma).reshape(1, CO), np.float32)
    be = np.ascontiguousarray(np.asarray(beta).reshape(1, CO), np.float32)
    xs = np.asarray(x).reshape(B, CT, 128, HWN).astype(bf)
    in_maps = []
    for b in range(B):
        in_maps.append({
            "x_bf": np.ascontiguousarray(xs[b]),
            "wconv": wconv, "wproj": wproj, "bias27": bias27,
            "gamma": ga, "beta": be,
        })
    return in_maps


def kernel(x, proj_w, proj_b, off_w, off_b, mask_w, mask_b, gamma, beta,
           _trace=False):
    n_cores = B
    nc = _get_nc(n_cores)
    in_maps = prep_inputs(x, proj_w, proj_b, off_w, off_b, mask_w, mask_b,
                          gamma, beta)
    res = run_bass_kernel_spmd(nc, in_maps, core_ids=list(range(n_cores)),
                               trace=_trace)
    outs = res.results if hasattr(res, "results") else res
    out = np.stack([np.asarray(outs[b]["out"]) for b in range(B)], axis=0)
    # [B, HW, CO] -> [B, CO, H, W] (pure layout, part of unshard)
    full = out.transpose(0, 2, 1).reshape(B, CO, H, W).astype(np.float32)
    if _trace:
        return full, res
    return full

